# revision 13
# baseline (speedup 1.0000x reference)
"""DeepseekV2 MLA attention (B=1, S=2048, H=4096, 32 heads) on 8 Trainium2
NeuronCores.

Sharding: tensor-parallel over heads (4 heads/core) for q_b/kv_b/o_w; the
small LoRA-A projections are data-parallel over sequence (each core computes
S/8 rows of q_lora / compressed-kv, rmsnorm'd locally, then on-device
AllGather). Final o-proj partials (row-parallel) are summed on the host as
the unshard step.

All matmuls run as float32r (TF32-like, 1 cycle/row on the PE at N>=256,
~1.5e-4 rel err per matmul). Softmax runs without max-subtraction (scores
are O(1) for this model family), with causal masking applied as a 0/1
multiply after exp. RoPE pairs are de-interleaved host-side (scores are
invariant to a shared permutation of q/k rope features), so the rotation is
pure elementwise work on [re... im...] blocks.
"""
import contextlib
import numpy as np

import concourse.bass as bass
import concourse.mybir as mybir
import concourse.tile as tile
from concourse import bacc
from concourse.bass_utils import run_bass_kernel_spmd

F32 = mybir.dt.float32
F32R = mybir.dt.float32r
I32 = mybir.dt.int32
AF = mybir.ActivationFunctionType
OP = mybir.AluOpType

P = 128
H = 4096
NH = 32
DN, DR, DV = 128, 64, 128
QK = DN + DR            # 192
RQ, RKV = 1536, 512
EPS = 1e-6
NCORES = 8
NHL = NH // NCORES      # 4 heads per core
SCALE = QK ** -0.5
TWO_PI = float(2 * np.pi)
HALF_PI = float(np.pi / 2)
PI = float(np.pi)

_BUILD_CACHE = {}


def build(S=2048):
    """Build the SPMD Bass program (identical on all 8 cores)."""
    R = S // NCORES          # rows (seq positions) per core in phase 1
    SB = S // 512            # 512-wide seq blocks
    KB = S // 128            # 128-wide seq blocks
    HKB = H // 128           # 32   hidden contraction blocks
    QMB = (NHL * QK) // 128  # 6    q_b output row-blocks
    RQB = RQ // 128          # 12
    RKB = RKV // 128         # 4
    NOB = H // 512           # 8    o-proj output col-blocks

    nc = bacc.Bacc("TRN2", target_bir_lowering=False, num_devices=NCORES)

    # ---- external inputs (host-prepped layouts; f32r = plain fp32 bits) ----
    hid_t = nc.declare_dram_parameter("hid_t", [HKB, P, R], F32R, isOutput=False)
    freqs_t = nc.declare_dram_parameter("freqs_t", [DR // 2, S], F32, isOutput=False)
    freqs_loc = nc.declare_dram_parameter("freqs_loc", [DR // 2, R], F32, isOutput=False)
    qaw_t = nc.declare_dram_parameter("qaw_t", [RQB, HKB, P, P], F32R, isOutput=False)
    qln = nc.declare_dram_parameter("qln", [P, RQB], F32, isOutput=False)
    kvaw_t = nc.declare_dram_parameter("kvaw_t", [RKB, HKB, P, P], F32R, isOutput=False)
    kvaw_rot = nc.declare_dram_parameter("kvaw_rot", [HKB, P, DR], F32R, isOutput=False)
    kvln = nc.declare_dram_parameter("kvln", [P, RKB], F32, isOutput=False)
    qbw_t = nc.declare_dram_parameter("qbw_t", [RQB, P, NHL * QK], F32R, isOutput=False)
    kvbw_t = nc.declare_dram_parameter("kvbw_t", [RKB, P, NHL * (DN + DV)], F32R, isOutput=False)
    ow_t = nc.declare_dram_parameter("ow_t", [NOB, NHL, P, 512], F32R, isOutput=False)
    mask_in = nc.declare_dram_parameter("mask", [P, 896], F32, isOutput=False)

    o_part = nc.declare_dram_parameter("o_part", [S, H], F32, isOutput=True)

    # ---- internal DRAM for the allgathers ----
    ag_ckv_in = nc.dram_tensor("ag_ckv_in", [RKV + DR, R], F32)
    ag_ckv_out = nc.dram_tensor("ag_ckv_out", [NCORES, RKV + DR, R], F32,
                                addr_space="Shared")
    ag_q_in = nc.dram_tensor("ag_q_in", [RQ, R], F32)
    ag_q_out = nc.dram_tensor("ag_q_out", [NCORES, RQ, R], F32,
                              addr_space="Shared")
    GROUPS = [list(range(NCORES))]

    with tile.TileContext(nc) as tc:
        _emit(nc, tc, locals())
    nc.compile()
    return nc


def _emit(nc, tc, ns):
    S = ns["S"]; R = ns["R"]; SB = ns["SB"]; KB = ns["KB"]; HKB = ns["HKB"]
    QMB = ns["QMB"]; RQB = ns["RQB"]; RKB = ns["RKB"]; NOB = ns["NOB"]
    hid_t = ns["hid_t"]; freqs_t = ns["freqs_t"]; freqs_loc = ns["freqs_loc"]
    qaw_t = ns["qaw_t"]; qln = ns["qln"]; kvaw_t = ns["kvaw_t"]
    kvaw_rot = ns["kvaw_rot"]; kvln = ns["kvln"]; qbw_t = ns["qbw_t"]
    kvbw_t = ns["kvbw_t"]; ow_t = ns["ow_t"]; mask_in = ns["mask_in"]
    o_part = ns["o_part"]
    ag_ckv_in = ns["ag_ckv_in"]; ag_ckv_out = ns["ag_ckv_out"]
    ag_q_in = ns["ag_q_in"]; ag_q_out = ns["ag_q_out"]; GROUPS = ns["GROUPS"]

    ctx = contextlib.ExitStack()
    with ctx:
        const = ctx.enter_context(tc.tile_pool(name="const", bufs=1))

        # constants
        qln_sb = const.tile([P, RQB], F32, tag="qln")
        nc.sync.dma_start(qln_sb[:], qln[:])
        kvln_sb = const.tile([P, RKB], F32, tag="kvln")
        nc.sync.dma_start(kvln_sb[:], kvln[:])
        mask_sb = const.tile([P, 896], F32, tag="mask")
        nc.sync.dma_start(mask_sb[:], mask_in[:])
        ones_f = const.tile([P, 1], F32, tag="onesf")
        nc.vector.memset(ones_f[:], 1.0)
        ones_col = const.tile([P, 1], F32R, tag="ones")
        nc.vector.tensor_copy(ones_col[:], ones_f[:])

        def build_cc(dst, src_dram, width, nrep, bias):
            """dst [32*nrep, width] = sin(freqs + bias), freqs replicated on
            nrep 32-partition blocks. bias=pi/2 yields cos. Range-reduced to
            [-pi, pi] (Sin LUT domain), robust to round-vs-trunc int casts."""
            with tc.tile_pool(name="ccb", bufs=1) as ccb:
                np_ = 32 * nrep
                f4 = ccb.tile([np_, width], F32, tag="f4")
                for k in range(nrep):
                    nc.sync.dma_start(f4[32 * k:32 * (k + 1), :], src_dram[:])
                if bias != 0.0:
                    nc.vector.tensor_scalar_add(f4[:], f4[:], bias)
                t0 = ccb.tile([np_, width], F32, tag="t0")
                ti = ccb.tile([np_, width], I32, tag="ti")
                nc.vector.tensor_scalar_mul(t0[:], f4[:], 1.0 / TWO_PI)
                nc.vector.tensor_copy(ti[:], t0[:])
                nc.vector.tensor_copy(t0[:], ti[:])
                nc.vector.tensor_scalar_mul(t0[:], t0[:], TWO_PI)
                nc.vector.tensor_sub(f4[:], f4[:], t0[:])
                nc.vector.tensor_scalar(t0[:], f4[:], PI, None, OP.is_gt)
                nc.vector.scalar_tensor_tensor(f4[:], t0[:], -TWO_PI, f4[:],
                                               OP.mult, OP.add)
                nc.scalar.activation(dst[:], f4[:], AF.Sin)

        # ---------------- phase 1: LoRA-A projections (rows of this core) ---
        with (
            tc.tile_pool(name="p1", bufs=2) as p1,
            tc.tile_pool(name="p1cp", bufs=1) as p1cp,
            tc.tile_pool(name="p1w", bufs=4) as p1w,
            tc.tile_pool(name="p1ps", bufs=2, space="PSUM") as p1ps,
            tc.tile_pool(name="p1ss", bufs=1, space="PSUM") as p1ss,
            tc.tile_pool(name="hidp", bufs=1) as hidp,
        ):
            # cos/sin tables for this core's positions, replicated on two
            # 32-partition blocks (so both the re [0:32] and im [32:64]
            # halves of k_rot can find an aligned operand).
            cosl = p1cp.tile([DR, R], F32, tag="cosl")
            build_cc(cosl, freqs_loc, R, 2, HALF_PI)
            sinl = p1cp.tile([DR, R], F32, tag="sinl")
            build_cc(sinl, freqs_loc, R, 2, 0.0)

            hid_all = hidp.tile([P, HKB, R], F32R, tag="hid")
            for kb in range(HKB):
                nc.sync.dma_start(hid_all[:, kb, :], hid_t[kb])

            def lora_a(n_mb, w_dram, ln_sb, rank, ag_in, rot_w):
                cp_all = p1cp.tile([P, n_mb, R], F32, tag=f"cp{n_mb}")
                ss_ps = p1ss.tile([1, R], F32, tag=f"ss{n_mb}")
                for mb in range(n_mb):
                    ps = p1ps.tile([P, R], F32, tag="p1ps")
                    for kb in range(HKB):
                        w = p1w.tile([P, P], F32R, tag="w")
                        nc.sync.dma_start(w[:], w_dram[mb, kb])
                        nc.tensor.matmul(ps[:], w[:], hid_all[:, kb, :],
                                         start=(kb == 0), stop=(kb == HKB - 1))
                    nc.scalar.copy(cp_all[:, mb, :], ps[:])
                    sq = p1.tile([P, R], F32R, tag="sq")
                    nc.vector.tensor_tensor(sq[:], cp_all[:, mb, :],
                                            cp_all[:, mb, :], OP.mult)
                    nc.tensor.matmul(ss_ps[:], ones_col[:], sq[:],
                                     start=(mb == 0), stop=(mb == n_mb - 1))
                # k_rot: [re(32); im(32)] raw -> rope -> ag rows [rank:rank+64]
                if rot_w is not None:
                    rps = p1ps.tile([DR, R], F32, tag="rps")
                    for kb in range(HKB):
                        w = p1w.tile([P, DR], F32R, tag="wr")
                        nc.sync.dma_start(w[:], rot_w[kb])
                        nc.tensor.matmul(rps[:], w[:], hid_all[:, kb, :],
                                         start=(kb == 0), stop=(kb == HKB - 1))
                    kr = p1.tile([DR, R], F32, tag="kr")
                    nc.vector.tensor_copy(kr[:], rps[:])
                    # products at their native bases
                    pa = p1.tile([DR, R], F32, tag="pa")
                    nc.vector.tensor_tensor(pa[0:32], kr[0:32], cosl[0:32], OP.mult)
                    nc.vector.tensor_tensor(pa[32:64], kr[32:64], sinl[32:64], OP.mult)
                    pb = p1.tile([DR, R], F32, tag="pb")
                    nc.vector.tensor_tensor(pb[0:32], kr[0:32], sinl[0:32], OP.mult)
                    nc.vector.tensor_tensor(pb[32:64], kr[32:64], cosl[32:64], OP.mult)
                    # partition-shift the im-half products down to base 0
                    sh_a = p1.tile([32, R], F32, tag="sha")
                    nc.sync.dma_start(sh_a[:], pa[32:64])
                    sh_b = p1.tile([32, R], F32, tag="shb")
                    nc.sync.dma_start(sh_b[:], pb[32:64])
                    out_r = p1.tile([32, R], F32, tag="outr")
                    nc.vector.tensor_sub(out_r[:], pa[0:32], sh_a[:])
                    out_i = p1.tile([32, R], F32, tag="outi")
                    nc.vector.tensor_add(out_i[:], pb[0:32], sh_b[:])
                    nc.sync.dma_start(ag_in[rank:rank + 32, :], out_r[:])
                    nc.sync.dma_start(ag_in[rank + 32:rank + DR, :], out_i[:])
                # rmsnorm: inv = 1/sqrt(ss/rank + eps), broadcast via gpsimd
                inv = p1.tile([1, R], F32, tag="inv")
                nc.vector.tensor_scalar(inv[:], ss_ps[:], 1.0 / rank, EPS,
                                        OP.mult, OP.add)
                nc.scalar.activation(inv[:], inv[:], AF.Sqrt)
                nc.vector.reciprocal(inv[:], inv[:])
                inv_bc = p1.tile([P, R], F32, tag="invbc")
                nc.gpsimd.partition_broadcast(inv_bc[:], inv[:])
                for mb in range(n_mb):
                    outn = p1.tile([P, R], F32, tag="outn")
                    nc.vector.scalar_tensor_tensor(
                        outn[:], cp_all[:, mb, :], ln_sb[:, mb:mb + 1],
                        inv_bc[:], OP.mult, OP.mult)
                    nc.sync.dma_start(ag_in[mb * P:(mb + 1) * P, :], outn[:])

            lora_a(RKB, kvaw_t, kvln_sb, RKV, ag_ckv_in, kvaw_rot)
            nc.gpsimd.collective_compute(
                "AllGather", OP.bypass, replica_groups=GROUPS,
                ins=[ag_ckv_in[:]], outs=[ag_ckv_out[:]])
            lora_a(RQB, qaw_t, qln_sb, RQ, ag_q_in, None)
            nc.gpsimd.collective_compute(
                "AllGather", OP.bypass, replica_groups=GROUPS,
                ins=[ag_q_in[:]], outs=[ag_q_out[:]])

        # ---------------- phase 2a: kv_b -> k_passT, v ----------------------
        kvres = ctx.enter_context(tc.tile_pool(name="kvres", bufs=1))
        kpass = [kvres.tile([P, S], F32R, tag=f"kp{h}", name=f"kp{h}")
                 for h in range(NHL)]
        krot_all = kvres.tile([DR, S], F32R, tag="krota")
        v_all = kvres.tile([P, KB, 512], F32R, tag="v")

        with (
            tc.tile_pool(name="ckvp", bufs=1) as ckvp,
            tc.tile_pool(name="p2ps", bufs=3, space="PSUM") as p2ps,
        ):
            ckv_sb = ckvp.tile([P, RKB, S], F32R, tag="ckv")
            for b in range(RKB):
                for r in range(NCORES):
                    nc.sync.dma_start(
                        ckv_sb[:, b, r * R:(r + 1) * R],
                        ag_ckv_out[r, b * P:(b + 1) * P, :].bitcast(F32R))
            for r in range(NCORES):
                nc.sync.dma_start(
                    krot_all[:, r * R:(r + 1) * R],
                    ag_ckv_out[r, RKV:RKV + DR, :].bitcast(F32R))
            kvbw = ckvp.tile([P, RKB, NHL * (DN + DV)], F32R, tag="kvbw")
            for b in range(RKB):
                nc.sync.dma_start(kvbw[:, b, :], kvbw_t[b])

            for h in range(NHL):
                for sb in range(SB):
                    ps = p2ps.tile([P, 512], F32, tag="ps2")
                    for b in range(RKB):
                        nc.tensor.matmul(
                            ps[:], kvbw[:, b, h * P:(h + 1) * P],
                            ckv_sb[:, b, sb * 512:(sb + 1) * 512],
                            start=(b == 0), stop=(b == RKB - 1))
                    nc.scalar.copy(kpass[h][:, sb * 512:(sb + 1) * 512], ps[:])
            for s in range(KB):
                ps = p2ps.tile([P, 512], F32, tag="ps2")
                for b in range(RKB):
                    nc.tensor.matmul(
                        ps[:], ckv_sb[:, b, s * P:(s + 1) * P],
                        kvbw[:, b, NHL * DN:],
                        start=(b == 0), stop=(b == RKB - 1))
                nc.scalar.copy(v_all[:, s, :], ps[:])

        # ---------------- phase 2b: q_b -> q_passT + packed rope ------------
        qres = ctx.enter_context(tc.tile_pool(name="qres", bufs=1))
        qpass = [qres.tile([P, S], F32R, tag=f"qp{h}", name=f"qp{h}")
                 for h in range(NHL)]
        q_re = qres.tile([P, S], F32R, tag="qre")   # [re_h0..re_h3] x32
        q_im = qres.tile([P, S], F32R, tag="qim")   # [im_h0..im_h3] x32

        with (
            tc.tile_pool(name="qbwp", bufs=1) as qbwp,
            tc.tile_pool(name="qlp", bufs=3) as qlp,
            tc.tile_pool(name="qbps", bufs=1, space="PSUM") as qbps,
        ):
            qbw = qbwp.tile([P, RQB, NHL * QK], F32R, tag="qbw")
            for kb in range(RQB):
                nc.sync.dma_start(qbw[:, kb, :], qbw_t[kb])
            for sb in range(SB):
                pss = [qbps.tile([P, 512], F32, tag=f"qps{m}", name=f"qps{m}")
                       for m in range(QMB)]
                for kb in range(RQB):
                    ql = qlp.tile([P, 512], F32R, tag="ql")
                    base = sb * 512
                    for half in range(512 // R if R <= 512 else 1):
                        r = (base + half * R) // R
                        nc.sync.dma_start(
                            ql[:, half * R:(half + 1) * R],
                            ag_q_out[r, kb * P:(kb + 1) * P, :].bitcast(F32R))
                    for m in range(QMB):
                        nc.tensor.matmul(
                            pss[m][:], qbw[:, kb, m * P:(m + 1) * P], ql[:],
                            start=(kb == 0), stop=(kb == RQB - 1))
                for m in range(QMB):
                    if m < NHL:
                        nc.scalar.copy(qpass[m][:, sb * 512:(sb + 1) * 512],
                                       pss[m][:])
                    elif m == NHL:
                        nc.scalar.copy(q_re[:, sb * 512:(sb + 1) * 512], pss[m][:])
                    else:
                        nc.scalar.copy(q_im[:, sb * 512:(sb + 1) * 512], pss[m][:])

        # packed rope over all 4 heads at once (full-128 ops, base 0)
        with (
            tc.tile_pool(name="ropeq", bufs=1) as ropeq,
            tc.tile_pool(name="ccfp", bufs=1) as ccfp,
        ):
            cos4 = ccfp.tile([P, S], F32, tag="cos4")
            build_cc(cos4, freqs_t, S, 4, HALF_PI)
            sin4 = ccfp.tile([P, S], F32, tag="sin4")
            build_cc(sin4, freqs_t, S, 4, 0.0)
            t = ropeq.tile([P, S], F32, tag="t")
            u = ropeq.tile([P, S], F32, tag="u")
            t2 = ropeq.tile([P, S], F32, tag="t2")
            qre_f = q_re[:].bitcast(F32)
            qim_f = q_im[:].bitcast(F32)
            nc.vector.tensor_tensor(t[:], qre_f, cos4[:], OP.mult)
            nc.vector.tensor_tensor(u[:], qim_f, sin4[:], OP.mult)
            nc.vector.tensor_tensor(t2[:], qre_f, sin4[:], OP.mult)
            nc.vector.tensor_sub(q_re[:], t[:], u[:])
            nc.vector.tensor_tensor(u[:], qim_f, cos4[:], OP.mult)
            nc.vector.tensor_add(q_im[:], t2[:], u[:])

        # ---------------- phase 3: attention --------------------------------
        ores = ctx.enter_context(tc.tile_pool(name="ores", bufs=1))
        o_heads = [ores.tile([P, S], F32R, tag=f"oh{h}", name=f"oh{h}")
                   for h in range(NHL)]

        with (
            tc.tile_pool(name="scps", bufs=3, space="PSUM") as scps,
            tc.tile_pool(name="ops", bufs=2, space="PSUM") as ops,
            tc.tile_pool(name="smps", bufs=1, space="PSUM") as smps,
            tc.tile_pool(name="att", bufs=2) as att,
            tc.tile_pool(name="attsp", bufs=2) as attsp,
        ):
            for qb in range(SB):
                q_sl = slice(qb * 512, (qb + 1) * 512)
                for h in range(NHL):
                    o_ps = ops.tile([P, 512], F32, tag="ops")
                    spart = attsp.tile([P, 512], F32R, tag="spart")
                    nkb = (qb + 1) * 4
                    # assemble this head's roped q_rot [re(32); im(32)]
                    qrs = att.tile([DR, 512], F32R, tag="qrs")
                    nc.sync.dma_start(qrs[0:32, :], q_re[32 * h:32 * (h + 1), q_sl])
                    nc.sync.dma_start(qrs[32:64, :], q_im[32 * h:32 * (h + 1), q_sl])
                    for kb in range(nkb):
                        k_sl = slice(kb * P, (kb + 1) * P)
                        s_ps = scps.tile([P, 512], F32, tag="sps")
                        nc.tensor.matmul(s_ps[:], kpass[h][:, k_sl],
                                         qpass[h][:, q_sl],
                                         start=True, stop=False)
                        nc.tensor.matmul(s_ps[:], krot_all[:, k_sl], qrs[:],
                                         start=False, stop=True)
                        probs = att.tile([P, 512], F32R, tag="probs")
                        d = kb * P - qb * 512
                        nc.scalar.activation(probs[:], s_ps[:], AF.Exp,
                                             scale=SCALE)
                        if d >= 0:
                            nc.vector.tensor_tensor(
                                probs[:], probs[:].bitcast(F32),
                                mask_sb[:, 384 - d:896 - d], OP.mult)
                        if kb == 0:
                            nc.vector.tensor_copy(spart[:], probs[:])
                        else:
                            nc.vector.tensor_tensor(
                                spart[:], spart[:].bitcast(F32),
                                probs[:].bitcast(F32), OP.add)
                        nc.tensor.matmul(o_ps[:], v_all[:, kb, h * P:(h + 1) * P],
                                         probs[:],
                                         start=(kb == 0), stop=(kb == nkb - 1))
                    sm_ps = smps.tile([1, 512], F32, tag="smps")
                    nc.tensor.matmul(sm_ps[:], ones_col[:], spart[:],
                                     start=True, stop=True)
                    rec = att.tile([1, 512], F32, tag="rec")
                    nc.vector.reciprocal(rec[:], sm_ps[:])
                    rec_bc = att.tile([P, 512], F32, tag="recbc")
                    nc.gpsimd.partition_broadcast(rec_bc[:], rec[:])
                    nc.vector.tensor_tensor(o_heads[h][:, q_sl], o_ps[:],
                                            rec_bc[:], OP.mult)

            # ------------- phase 4: o-proj ----------------------------------
            with (
                tc.tile_pool(name="owp", bufs=1) as owp,
                tc.tile_pool(name="oj", bufs=2) as oj,
                tc.tile_pool(name="ojps", bufs=2, space="PSUM") as ojps,
            ):
                for nb in range(NOB):
                    ow = owp.tile([P, NHL, 512], F32R, tag="ow")
                    for h in range(NHL):
                        nc.sync.dma_start(ow[:, h, :], ow_t[nb, h])
                    for s in range(KB):
                        ps = ojps.tile([P, 512], F32, tag="ojps")
                        for h in range(NHL):
                            nc.tensor.matmul(
                                ps[:], o_heads[h][:, s * P:(s + 1) * P],
                                ow[:, h, :],
                                start=(h == 0), stop=(h == NHL - 1))
                        ot = oj.tile([P, 512], F32, tag="ot")
                        nc.vector.tensor_copy(ot[:], ps[:])
                        nc.sync.dma_start(
                            o_part[s * P:(s + 1) * P, nb * 512:(nb + 1) * 512],
                            ot[:])


# ======================= host-side prep & entry ==========================

def _deinterleave(rows):
    """Reorder rope rows from interleaved (re,im,re,im,...) to [re... im...]."""
    return np.concatenate([rows[0::2], rows[1::2]], axis=0)


def prep_inputs(hidden_states, freqs, q_a_w, q_a_ln_w, q_b_w, kv_a_w,
                kv_a_ln_w, kv_b_w, o_w):
    S = hidden_states.shape[1]
    R = S // NCORES
    f32 = np.float32

    hidT = np.ascontiguousarray(hidden_states[0].T.astype(f32))      # [H, S]
    freqsT = np.ascontiguousarray(freqs[0].T.astype(f32))            # [32, S]

    qawT = q_a_w.astype(f32).T                                       # [H, RQ]
    qaw_t = np.ascontiguousarray(
        qawT.reshape(H // P, P, RQ // P, P).transpose(2, 0, 1, 3))
    qln_h = np.ascontiguousarray(q_a_ln_w.astype(f32).reshape(RQ // P, P).T)

    kva = kv_a_w.astype(f32)
    kva_main = kva[:RKV]
    kva_rot = _deinterleave(kva[RKV:])
    kvaw_t = np.ascontiguousarray(
        kva_main.T.reshape(H // P, P, RKV // P, P).transpose(2, 0, 1, 3))
    kvaw_rot_t = np.ascontiguousarray(kva_rot.T.reshape(H // P, P, DR))
    kvln_h = np.ascontiguousarray(kv_a_ln_w.astype(f32).reshape(RKV // P, P).T)

    # big causal mask [128, 896]: M[p, g] = 1 if g >= p + 384
    g = np.arange(896)[None, :]
    p = np.arange(P)[:, None]
    mask = (g >= p + 384).astype(f32)

    qbw = q_b_w.astype(f32)
    kvbw = kv_b_w.astype(f32)
    ow = o_w.astype(f32)

    in_maps = []
    for c in range(NCORES):
        heads = list(range(NHL * c, NHL * (c + 1)))
        # q_b rows: [pass_h0..h3 | re_h0..h3 | im_h0..h3]
        qb_pass = np.concatenate([qbw[QK * h:QK * h + DN] for h in heads], axis=0)
        qb_re = np.concatenate(
            [qbw[QK * h + DN:QK * (h + 1)][0::2] for h in heads], axis=0)
        qb_im = np.concatenate(
            [qbw[QK * h + DN:QK * (h + 1)][1::2] for h in heads], axis=0)
        qb_core = np.concatenate([qb_pass, qb_re, qb_im], axis=0)    # [768, RQ]
        qbw_t = np.ascontiguousarray(qb_core.T.reshape(RQ // P, P, NHL * QK))
        # kv_b rows: [kpass_h0..h3 | v_h0..h3]
        kp = np.concatenate(
            [kvbw[(DN + DV) * h:(DN + DV) * h + DN] for h in heads], axis=0)
        vv = np.concatenate(
            [kvbw[(DN + DV) * h + DN:(DN + DV) * (h + 1)] for h in heads], axis=0)
        kvb_core = np.concatenate([kp, vv], axis=0)                  # [1024, RKV]
        kvbw_tc = np.ascontiguousarray(
            kvb_core.T.reshape(RKV // P, P, NHL * (DN + DV)))
        # o_w columns for this core's heads, transposed, tiled [8,4,128,512]
        ow_slice = ow[:, NHL * DV * c: NHL * DV * (c + 1)].T         # [512, H]
        ow_tc = np.ascontiguousarray(
            ow_slice.reshape(NHL, P, H // 512, 512).transpose(2, 0, 1, 3))

        hid_c = np.ascontiguousarray(
            hidT[:, R * c:R * (c + 1)].reshape(H // P, P, R))
        freqs_c = np.ascontiguousarray(freqsT[:, R * c:R * (c + 1)])

        in_maps.append({
            "hid_t": hid_c,
            "freqs_t": freqsT,
            "freqs_loc": freqs_c,
            "qaw_t": qaw_t,
            "qln": qln_h,
            "kvaw_t": kvaw_t,
            "kvaw_rot": kvaw_rot_t,
            "kvln": kvln_h,
            "qbw_t": qbw_t,
            "kvbw_t": kvbw_tc,
            "ow_t": ow_tc,
            "mask": mask,
        })
    return in_maps


def _run(inputs, trace=False, trace_kwargs=None):
    S = inputs["hidden_states"].shape[1]
    if S not in _BUILD_CACHE:
        _BUILD_CACHE[S] = build(S)
    nc = _BUILD_CACHE[S]
    in_maps = prep_inputs(**inputs)
    kw = {}
    if trace:
        kw["trace"] = True
        if trace_kwargs:
            kw.update(trace_kwargs)
    res = run_bass_kernel_spmd(nc, in_maps, list(range(NCORES)), **kw)
    parts = np.stack([r["o_part"] for r in res.results], axis=0)
    out = parts.sum(axis=0, dtype=np.float64).astype(np.float32)
    return out[None], res


def kernel(**inputs):
    out, _ = _run(inputs)
    return out
